# revision 2
# baseline (speedup 1.0000x reference)
"""Butterfly depthwise-conv kernel for 8 Trainium2 NeuronCores.

Sharding: data-parallel over batch (B=8 -> 1 sample per core). Inside a core:
partitions = (channel, H-half): p<64 -> channel p rows [0, H/2); p>=64 ->
channel p-64 rows [H/2, H). Free axis = padded rows of one half:
row stride W+2, 2 left-pad cols, 1 halo row above/below the interior
(plus one extra zero guard row so the dx=+1 tap's row-wrap read of the
bottom halo row stays in bounds).

Per stage: the 9 depthwise taps are k=64 matmuls whose lhsT packs BOTH
branches (m=128: cols 0-63 branch a diag(w0), cols 64-127 branch b with the
butterfly mask folded in). Half-A and half-B matmuls run concurrently on
disjoint PE row-groups. PSUM accumulates the 9 taps; eviction does
relu(psum + bias) per branch (ACT/DVE, using cross-partition PSUM reads)
and a partition-aligned DVE add. Stage 5 fuses the +x residual and streams
the fp32 result to HBM.
"""

import json
import sys

sys.path.insert(0, "/opt/trn_rl_repo")

import numpy as np

import concourse.bass as bass
import concourse.mybir as mybir
from concourse.tile import TileContext
from concourse.bass_utils import run_bass_kernel_spmd

# ---------------------------------------------------------------------------
# Walrus in this container accepts at most ONE sem wait / update per
# instruction; Tile emits more. Rewrite the BIR JSON before serialization:
# hoist excess waits onto preceding same-engine NoOps and excess updates onto
# trailing same-engine NoOps (engine queues are FIFO; a NoOp's update fires
# after the preceding instruction completes).
_wsplit_counter = [0]


def _fresh_name():
    _wsplit_counter[0] += 1
    return f"I-wsplit-{_wsplit_counter[0]}"


def _nop(engine, debug, wait=None, update=None):
    return {
        "debug": debug,
        "engine": engine,
        "ins": [],
        "name": _fresh_name(),
        "opcode": "NoOp",
        "outs": [],
        "sync_info": {
            "on_update": [update] if update else [],
            "on_wait": [wait] if wait else [],
        },
    }


def _rewrite_bir(j):
    for fn in j["functions"]:
        for bb in fn["blocks"]:
            new_insts = []
            for inst in bb["instructions"]:
                si = inst.get("sync_info")
                pre, post = [], []
                if si:
                    waits = si.get("on_wait") or []
                    if len(waits) > 1:
                        for w in waits[:-1]:
                            pre.append(_nop(inst["engine"], inst.get("debug", 0), wait=w))
                        si["on_wait"] = [waits[-1]]
                    ups = si.get("on_update") or []
                    opc = inst.get("opcode", "")
                    if len(ups) > 1 and "DMA" not in opc and "Dma" not in opc:
                        for u in ups[1:]:
                            post.append(_nop(inst["engine"], inst.get("debug", 0), update=u))
                        si["on_update"] = ups[:1]
                new_insts.extend(pre)
                new_insts.append(inst)
                new_insts.extend(post)
            bb["instructions"] = new_insts
    return j


_orig_to_json_bytes = bass.Bass.to_json_bytes


def _patched_to_json_bytes(self, *a, **kw):
    raw = _orig_to_json_bytes(self, *a, **kw)
    return json.dumps(_rewrite_bir(json.loads(raw))).encode()


bass.Bass.to_json_bytes = _patched_to_json_bytes
# ---------------------------------------------------------------------------

C = 64
AF = mybir.ActivationFunctionType
ALU = mybir.AluOpType


def build_program(H, W, num_bf):
    """Emit the Bass program for one core (one batch sample)."""
    HALF = H // 2
    SW = W + 2  # padded row stride
    ROWS = HALF + 2  # interior + top/bottom halo rows
    L = (ROWS + 1) * SW  # + one zero guard row for the corner wrap read
    CPR = 512 // W  # interior rows per psum chunk (2 at W=256)
    GROUP_ROWS = 2 * CPR  # rows per evict group (2 chunks)
    n_groups = HALF // GROUP_ROWS
    assert HALF % GROUP_ROWS == 0
    NCOL = GROUP_ROWS * W  # eviction columns per group (1024 at W=256)

    nc = bass.Bass()
    x_ext = nc.declare_dram_parameter("x", [C, H, W], mybir.dt.float32, isOutput=False)
    wt_ext = nc.declare_dram_parameter(
        "lhsT", [num_bf * 9, C, 128], mybir.dt.float32, isOutput=False
    )
    bias_ext = nc.declare_dram_parameter(
        "bias", [128, num_bf], mybir.dt.float32, isOutput=False
    )
    out_ext = nc.declare_dram_parameter("out", [C, H, W], mybir.dt.float32, isOutput=True)

    def interior(r):
        # free-axis element offset of interior row r (0-based), col 0
        return (r + 1) * SW + 2

    def rows_ap(tile, pslice, r0, nrows, base_off=0):
        """[pslice, nrows, W] view of interior rows r0..r0+nrows-1 (+base_off cols)."""
        o = interior(r0) + base_off
        v = tile[pslice, o : o + nrows * SW]
        return v.rearrange("p (r w) -> p r w", w=SW)[:, :, 0:W]

    with TileContext(nc) as tc:
        with (
            tc.tile_pool(name="state", bufs=1) as state,
            tc.tile_pool(name="evict", bufs=2) as evict,
            tc.tile_pool(name="res", bufs=2) as res,
            tc.tile_pool(name="psA", bufs=2, space="PSUM") as psum_a,
            tc.tile_pool(name="psB", bufs=2, space="PSUM") as psum_b,
        ):
            now0 = state.tile([128, L], mybir.dt.bfloat16)
            now1 = state.tile([128, L], mybir.dt.bfloat16)
            wt = state.tile([128, num_bf * 9 * 128], mybir.dt.bfloat16)
            bias_t = state.tile([128, num_bf], mybir.dt.float32)

            # zero both state buffers (pad cols + halo rows stay 0 forever)
            nc.vector.memset(now0[:], 0.0)
            nc.vector.memset(now1[:], 0.0)

            # weights: same data on partitions 0-63 and 64-127
            wt_src = wt_ext[:, :, :].rearrange("t c m -> c t m")
            nc.gpsimd.dma_start(
                out=wt[0:64, :].rearrange("p (t m) -> p t m", m=128), in_=wt_src
            )
            nc.gpsimd.dma_start(
                out=wt[64:128, :].rearrange("p (t m) -> p t m", m=128), in_=wt_src
            )
            nc.sync.dma_start(out=bias_t[:], in_=bias_ext[:])

            # initial load: x fp32 -> now0 bf16, both halves + boundary halo rows
            for h_base, pslice in ((0, slice(0, 64)), (HALF, slice(64, 128))):
                nsplit = 4
                for part in range(nsplit):
                    r0 = part * (HALF // nsplit)
                    r1 = (part + 1) * (HALF // nsplit)
                    nc.gpsimd.dma_start(
                        out=rows_ap(now0, pslice, r0, r1 - r0),
                        in_=x_ext[:, h_base + r0 : h_base + r1, :],
                    )
            bot = now0[0:64, interior(HALF) : interior(HALF) + W]
            nc.gpsimd.dma_start(out=bot, in_=x_ext[:, HALF : HALF + 1, :])
            top = now0[64:128, interior(-1) : interior(-1) + W]
            nc.gpsimd.dma_start(out=top, in_=x_ext[:, HALF - 1 : HALF, :])

            bufs = [now0, now1]
            for i in range(num_bf):
                src = bufs[i % 2]
                dst = bufs[(i + 1) % 2]
                last = i == num_bf - 1
                ba = bias_t[0:64, i : i + 1]
                bb = bias_t[64:128, i : i + 1]
                for g in range(n_groups):
                    ps_a = psum_a.tile([128, 1024], mybir.dt.float32)
                    ps_b = psum_b.tile([128, 1024], mybir.dt.float32)
                    for cp in range(2):
                        r0 = g * GROUP_ROWS + cp * CPR
                        for t in range(9):
                            dy, dx = divmod(t, 3)
                            dy -= 1
                            dx -= 1
                            wslice = wt[:, (i * 9 + t) * 128 : (i * 9 + t + 1) * 128]
                            for ps, pslice in ((ps_a, slice(0, 64)), (ps_b, slice(64, 128))):
                                rhs = rows_ap(src, pslice, r0 + dy, CPR, base_off=dx)
                                po = ps[:, cp * 512 : (cp + 1) * 512]
                                po = po.rearrange("p (r w) -> p r w", w=W)
                                nc.tensor.matmul(
                                    po,
                                    wslice[pslice, :],
                                    rhs,
                                    start=(t == 0),
                                    stop=(t == 8),
                                )
                    # ---- eviction of GROUP_ROWS rows per half ----
                    ua_a = evict.tile([64, NCOL], mybir.dt.bfloat16, tag="ua_a")
                    ub_a = evict.tile([64, NCOL], mybir.dt.bfloat16, tag="ub_a")
                    ua_b = evict.tile([128, NCOL], mybir.dt.bfloat16, tag="ua_b")
                    ub_b = evict.tile([128, NCOL], mybir.dt.bfloat16, tag="ub_b")
                    # half A: branch partials at psA[0:64] (a) and psA[64:128] (b)
                    nc.scalar.activation(ua_a[:, :], ps_a[0:64, 0:NCOL], AF.Relu, bias=ba, scale=1.0)
                    nc.scalar.activation(ub_a[:, :], ps_a[64:128, 0:NCOL], AF.Relu, bias=bb, scale=1.0)
                    # half B: place relu'd branches at partitions 64:128
                    nc.scalar.activation(ua_b[64:128, :], ps_b[0:64, 0:NCOL], AF.Relu, bias=ba, scale=1.0)
                    nc.vector.tensor_scalar(
                        out=ub_b[64:128, :],
                        in0=ps_b[64:128, 0:NCOL],
                        scalar1=bb,
                        scalar2=0.0,
                        op0=ALU.add,
                        op1=ALU.max,
                    )
                    r0 = g * GROUP_ROWS
                    uaa3 = ua_a[:, :].rearrange("p (r w) -> p r w", w=W)
                    uba3 = ub_a[:, :].rearrange("p (r w) -> p r w", w=W)
                    uab3 = ua_b[64:128, :].rearrange("p (r w) -> p r w", w=W)
                    ubb3 = ub_b[64:128, :].rearrange("p (r w) -> p r w", w=W)
                    if not last:
                        nc.vector.tensor_add(rows_ap(dst, slice(0, 64), r0, GROUP_ROWS), uaa3, uba3)
                        nc.vector.tensor_add(rows_ap(dst, slice(64, 128), r0, GROUP_ROWS), uab3, ubb3)
                    else:
                        # final stage: y + x residual, fp32 out
                        y_a = evict.tile([64, NCOL], mybir.dt.bfloat16, tag="y_a")
                        y_b = evict.tile([128, NCOL], mybir.dt.bfloat16, tag="y_b")
                        nc.vector.tensor_add(y_a[:, :], ua_a[:, :], ub_a[:, :])
                        nc.vector.tensor_add(y_b[64:128, :], ua_b[64:128, :], ub_b[64:128, :])
                        xr = res.tile([128, NCOL], mybir.dt.float32, tag="xr")
                        og = res.tile([128, NCOL], mybir.dt.float32, tag="og")
                        nc.sync.dma_start(
                            out=xr[0:64, :].rearrange("p (r w) -> p r w", w=W),
                            in_=x_ext[:, r0 : r0 + GROUP_ROWS, :],
                        )
                        nc.sync.dma_start(
                            out=xr[64:128, :].rearrange("p (r w) -> p r w", w=W),
                            in_=x_ext[:, HALF + r0 : HALF + r0 + GROUP_ROWS, :],
                        )
                        nc.vector.tensor_add(og[0:64, :], y_a[:, :], xr[0:64, :])
                        nc.vector.tensor_add(og[64:128, :], y_b[64:128, :], xr[64:128, :])
                        nc.sync.dma_start(
                            out=out_ext[:, r0 : r0 + GROUP_ROWS, :],
                            in_=og[0:64, :].rearrange("p (r w) -> p r w", w=W),
                        )
                        nc.sync.dma_start(
                            out=out_ext[:, HALF + r0 : HALF + r0 + GROUP_ROWS, :],
                            in_=og[64:128, :].rearrange("p (r w) -> p r w", w=W),
                        )
                if not last:
                    # half-boundary halo exchange inside dst
                    nc.sync.dma_start(
                        out=dst[64:128, interior(-1) : interior(-1) + W],
                        in_=dst[0:64, interior(HALF - 1) : interior(HALF - 1) + W],
                    )
                    nc.sync.dma_start(
                        out=dst[0:64, interior(HALF) : interior(HALF) + W],
                        in_=dst[64:128, interior(0) : interior(0) + W],
                    )
    return nc


def host_prep(weights, biases, masks, num_bf):
    """Fold the butterfly masks into per-tap lhsT matrices."""
    lhsT = np.zeros((num_bf * 9, C, 128), dtype=np.float32)
    for i in range(num_bf):
        m = masks[i]
        for t in range(9):
            dy, dx = divmod(t, 3)
            for c in range(C):
                lhsT[i * 9 + t, c, c] = weights[i, 0, c, 0, dy, dx]
                lhsT[i * 9 + t, m[c], 64 + c] = weights[i, 1, c, 0, dy, dx]
    bias = np.concatenate([biases[:, 0, :], biases[:, 1, :]], axis=1)  # [nb, 128]
    bias = np.ascontiguousarray(bias.T.astype(np.float32))  # [128, nb]
    return lhsT, bias


def _run(x_full, weights, biases, masks, H, W, num_bf, trace=False):
    nc = build_program(H, W, num_bf)
    lhsT, bias = host_prep(
        np.asarray(weights, dtype=np.float32),
        np.asarray(biases, dtype=np.float32),
        np.asarray(masks),
        num_bf,
    )
    n = x_full.shape[0]
    in_maps = [
        {"x": np.ascontiguousarray(x_full[b], dtype=np.float32), "lhsT": lhsT, "bias": bias}
        for b in range(n)
    ]
    r = run_bass_kernel_spmd(nc, in_maps, core_ids=list(range(n)), trace=trace)
    out = np.stack([r.results[b]["out"] for b in range(n)], axis=0)
    return out, r


def kernel(x, weights, biases, masks):
    x = np.asarray(x, dtype=np.float32)
    out, _ = _run(x, weights, biases, masks, H=256, W=256, num_bf=6)
    return out
